# revision 6
# baseline (speedup 1.0000x reference)
"""Trainium2 Bass kernel for nn_AssociationLayer (batched masked Sinkhorn).

Self-contained: kernel(**inputs) takes the FULL unsharded inputs
(affinity_scores [256,256,256] f32, num_detections [256] i32,
num_tracklets [256] i32) and returns (sinkhorn_dense [256, 65536] f32,
assignment_dense [256, 65536] bool), matching the reference.

Distribution: pure data parallelism — batch is sharded 8 x 32 across the 8
NeuronCores; each core solves its 32 examples independently (no collectives).

Per-core algorithm (groups of 4 examples):
  A = exp(10*aff), stored as A_hi + A_lo (two bf16 tiles, ~2^-17 precision)
  95 main iterations: bf16 matvecs on TensorE (4-way column-packed via
    tile_position), denominators moved PSUM->SBUF by ScalarE (+border bias),
    transposed back to weight layout by TensorE, reciprocal+mask on VectorE
  5 tail iterations: 3-term split-bf16 matvecs (f32-grade precision)
  epilogue: transport T = (u_i*A_ij)*v_j, mutual row/col argmax assignment
  Ragged pack to the reference's flattened layout happens on host.
"""
import numpy as np
from contextlib import ExitStack

import concourse.bass as bass
import concourse.bacc as bacc
import concourse.mybir as mybir
from concourse import tile
from concourse import bass_utils
from concourse.masks import make_identity
import concourse.bass_isa as bass_isa

F32 = mybir.dt.float32
BF16 = mybir.dt.bfloat16
U8 = mybir.dt.uint8
AF = mybir.ActivationFunctionType
ALU = mybir.AluOpType
AX = mybir.AxisListType

N_CORES = 8
B, MT, MD = 256, 256, 256
N_EX = B // N_CORES              # 32 examples per core
N_ITERS = 100
N_TAIL = 5


def _build(n_ex=N_EX, n_iters=N_ITERS, n_tail=N_TAIL, static=False, staggered=False):
    assert n_ex % 4 == 0
    G = n_ex // 4
    n_main = n_iters - n_tail
    nc = bacc.Bacc("TRN2", target_bir_lowering=False, debug=False)

    aff = nc.dram_tensor("aff", [n_ex, 128, 512], F32, kind="ExternalInput").ap()
    afft = nc.dram_tensor("afft", [n_ex, 128, 512], F32, kind="ExternalInput").ap()
    rt01c = nc.dram_tensor("rt01c", [G, 128, 8], F32, kind="ExternalInput").ap()
    ct01c = nc.dram_tensor("ct01c", [G, 128, 8], F32, kind="ExternalInput").ap()
    ntf = nc.dram_tensor("ntf", [G, 128, 1], F32, kind="ExternalInput").ap()
    ndf = nc.dram_tensor("ndf", [G, 128, 1], F32, kind="ExternalInput").ap()
    ct01r = nc.dram_tensor("ct01r", [G, 128, 256], F32, kind="ExternalInput").ap()
    ones_in = nc.dram_tensor("ones_in", [128, 1], F32, kind="ExternalInput").ap()
    t_out = nc.dram_tensor("t_out", [n_ex, 128, 512], F32, kind="ExternalOutput").ap()
    a_out = nc.dram_tensor("a_out", [n_ex, 128, 512], U8, kind="ExternalOutput").ap()

    with tile.TileContext(nc) as tc, ExitStack() as ctx:
        P = ctx.enter_context
        const = P(tc.tile_pool(name="const", bufs=1))
        mats = P(tc.tile_pool(name="mats", bufs=1))
        state = P(tc.tile_pool(name="state", bufs=1))
        pp_mv = P(tc.tile_pool(name="pp_mv", bufs=1, space="PSUM"))
        pp_tp = P(tc.tile_pool(name="pp_tp", bufs=1, space="PSUM"))
        pp_sm = P(tc.tile_pool(name="pp_sm", bufs=1, space="PSUM"))

        ident_f = const.tile([128, 128], F32, tag="ident_f")
        ident_b = const.tile([128, 128], BF16, tag="ident_b")
        make_identity(nc, ident_f[:])
        nc.vector.tensor_copy(ident_b[:], ident_f[:])
        ones_f = const.tile([128, 1], F32, tag="ones_f")
        ones_b = const.tile([128, 1], BF16, tag="ones_b")
        nc.sync.dma_start(ones_f[:], ones_in[:])
        nc.vector.tensor_copy(ones_b[:], ones_f[:])

        rt01c_sb = const.tile([128, G * 8], F32, tag="rt01c")
        ct01c_sb = const.tile([128, G * 8], F32, tag="ct01c")
        ntf_sb = const.tile([128, G], F32, tag="ntf")
        ndf_sb = const.tile([128, G], F32, tag="ndf")
        for q in range(G):
            nc.sync.dma_start(rt01c_sb[:, q * 8:(q + 1) * 8], rt01c[q])
            nc.sync.dma_start(ct01c_sb[:, q * 8:(q + 1) * 8], ct01c[q])
            nc.sync.dma_start(ntf_sb[:, q:q + 1], ntf[q])
            nc.sync.dma_start(ndf_sb[:, q:q + 1], ndf[q])

        A_hi = mats.tile([128, n_ex * 512], BF16, tag="A_hi")
        A_lo = mats.tile([128, n_ex * 512], BF16, tag="A_lo")
        AT_hi = mats.tile([128, n_ex * 512], BF16, tag="AT_hi")
        AT_lo = mats.tile([128, n_ex * 512], BF16, tag="AT_lo")
        with tc.tile_pool(name="stage", bufs=3) as stg:
            for e in range(n_ex):
                for src, hi, lo in ((aff, A_hi, A_lo), (afft, AT_hi, AT_lo)):
                    sl = slice(e * 512, (e + 1) * 512)
                    st = stg.tile([128, 512], F32, tag="ldstage", name="ldstage")
                    nc.sync.dma_start(st[:], src[e])
                    af = stg.tile([128, 512], F32, tag="af32", name="af32")
                    nc.scalar.activation(af[:], st[:], AF.Exp, bias=0.0, scale=10.0)
                    nc.scalar.activation(hi[:, sl], af[:], AF.Copy, bias=0.0, scale=1.0)
                    nc.vector.tensor_tensor(lo[:, sl], af[:], hi[:, sl], ALU.subtract)

        def tiles(shape, dt_, pfx):
            return [state.tile(shape, dt_, tag=f"{pfx}{q}", name=f"{pfx}{q}")
                    for q in range(G)]
        u_col = tiles([128, 8], BF16, "u")
        v_col = tiles([128, 8], BF16, "v")
        uf = tiles([128, 8], F32, "uf")
        vf = tiles([128, 8], F32, "vf")
        u_lo = tiles([128, 8], BF16, "ulo")
        v_lo = tiles([128, 8], BF16, "vlo")
        den_ub = tiles([128, 256], BF16, "dub")
        den_vb = tiles([128, 256], BF16, "dvb")
        den_uf = tiles([128, 256], F32, "duf")
        den_vf = tiles([128, 256], F32, "dvf")
        Su = tiles([128, 1], F32, "Su")
        Sv = tiles([128, 1], F32, "Sv")
        ub = tiles([128, 1], F32, "ub")
        vb = tiles([128, 1], F32, "vb")
        rec8 = tiles([128, 8], F32, "r8")
        dbt = tiles([128, 1], F32, "db")

        NPM, NTP, NSM = 3, 3, 2
        pmv = [pp_mv.tile([128, 256], F32, tag=f"pmv{i}", name=f"pmv{i}")
               for i in range(NPM)]
        ptp = [pp_tp.tile([128, 256], F32, tag=f"ptp{i}", name=f"ptp{i}")
               for i in range(NTP)]
        psm = [pp_sm.tile([128, 2], F32, tag=f"psm{i}", name=f"psm{i}")
               for i in range(NSM)]
        for t_ in pmv + ptp + psm:
            nc.vector.memset(t_[:, :], 0.0)

        for q in range(G):
            nc.vector.tensor_copy(v_col[q][:], ct01c_sb[:, q * 8:(q + 1) * 8])
            nc.vector.memset(vb[q][:], 1.0)
            nc.vector.tensor_copy(Sv[q][:], ndf_sb[:, q:q + 1])

        def wslice(col, g, t):
            return col[:, :].rearrange("p (t g) -> p g t", g=4)[:, g, t:t + 1]

        def sslice(col, g):
            return col[:, :].rearrange("p (t g) -> p g t", g=4)[:, g, :]

        def border(q, S_cur, b_cur, b_new, nf_sb):
            nc.vector.tensor_add(dbt[q][:], S_cur[:], b_cur[:])
            nc.vector.reciprocal(dbt[q][:], dbt[q][:])
            nc.vector.tensor_mul(b_new[:], nf_sb, dbt[q][:])

        def half_bf16(q, mat, w_col, out_col, mask_sb, den_sb, S_new, S_cur,
                      b_cur, b_new, nf_sb):
            pm, pt, ps = pmv[q % NPM], ptp[q % NTP], psm[q % NSM]
            for g in range(4):
                e = q * 4 + g
                for t in range(2):
                    nc.tensor.matmul(
                        pm[32 * g:32 * g + 1, :], wslice(w_col, g, t),
                        mat[:, e * 512 + t * 256: e * 512 + (t + 1) * 256],
                        start=(t == 0), stop=(t == 1),
                        tile_position=(0, 32 * g), skip_group_check=True)
            border(q, S_cur, b_cur, b_new, nf_sb)
            nc.scalar.activation(den_sb[:], pm[:, :], AF.Identity,
                                 bias=b_cur[:, 0:1], scale=1.0)
            ptb = pt[:, :].bitcast(BF16)
            nc.tensor.transpose(ptb[:, 0:128], den_sb[:, 0:128], ident_b[:])
            nc.tensor.transpose(ptb[:, 128:256], den_sb[:, 128:256], ident_b[:])
            tp_src = ptb[:, 0:256].rearrange("p (t g r) -> p t g r", t=2, g=4)[:, :, :, 0]
            nc.vector.reciprocal(rec8[q][:].rearrange("p (t g) -> p t g", t=2), tp_src)
            nc.vector.tensor_mul(out_col[:], rec8[q][:], mask_sb)
            for g in range(4):
                nc.tensor.matmul(ps[32 * g:32 * g + 1, :], ones_b[:],
                                 sslice(out_col, g), start=True, stop=True,
                                 tile_position=(0, 32 * g), skip_group_check=True)
            nc.vector.tensor_reduce(S_new[:], ps[:, :], axis=AX.X, op=ALU.add)

        def half_tail(q, mat_hi, mat_lo, whi, wlo, out_f, out_hi, out_lo,
                      mask_sb, den_sb, S_new, S_cur, b_cur, b_new, nf_sb):
            pm, pt, ps = pmv[q % NPM], ptp[q % NTP], psm[q % NSM]
            for g in range(4):
                e = q * 4 + g
                for t in range(2):
                    msl = slice(e * 512 + t * 256, e * 512 + (t + 1) * 256)
                    nc.tensor.matmul(pm[32 * g:32 * g + 1, :], wslice(whi, g, t),
                                     mat_hi[:, msl], start=(t == 0), stop=False,
                                     tile_position=(0, 32 * g), skip_group_check=True)
                    nc.tensor.matmul(pm[32 * g:32 * g + 1, :], wslice(wlo, g, t),
                                     mat_hi[:, msl], start=False, stop=False,
                                     tile_position=(0, 32 * g), skip_group_check=True)
                    nc.tensor.matmul(pm[32 * g:32 * g + 1, :], wslice(whi, g, t),
                                     mat_lo[:, msl], start=False, stop=(t == 1),
                                     tile_position=(0, 32 * g), skip_group_check=True)
            border(q, S_cur, b_cur, b_new, nf_sb)
            nc.scalar.activation(den_sb[:], pm[:, :], AF.Identity,
                                 bias=b_cur[:, 0:1], scale=1.0)
            nc.tensor.transpose(pt[:, 0:128], den_sb[:, 0:128], ident_f[:])
            nc.tensor.transpose(pt[:, 128:256], den_sb[:, 128:256], ident_f[:])
            tp_src = pt[:, 0:256].rearrange("p (t g r) -> p t g r", t=2, g=4)[:, :, :, 0]
            nc.vector.reciprocal(rec8[q][:].rearrange("p (t g) -> p t g", t=2), tp_src)
            nc.vector.tensor_mul(out_f[:], rec8[q][:], mask_sb)
            nc.scalar.activation(out_hi[:], out_f[:], AF.Copy, bias=0.0, scale=1.0)
            nc.vector.tensor_tensor(out_lo[:], out_f[:], out_hi[:], ALU.subtract)
            for g in range(4):
                nc.tensor.matmul(ps[32 * g:32 * g + 1, :], ones_f[:],
                                 sslice(out_f, g), start=True, stop=True,
                                 tile_position=(0, 32 * g), skip_group_check=True)
            nc.vector.tensor_reduce(S_new[:], ps[:, :], axis=AX.X, op=ALU.add)

        def u_half(q):
            half_bf16(q, AT_hi, v_col[q], u_col[q],
                      rt01c_sb[:, q * 8:(q + 1) * 8], den_ub[q],
                      Su[q], Sv[q], vb[q], ub[q], ntf_sb[:, q:q + 1])

        def v_half(q):
            half_bf16(q, A_hi, u_col[q], v_col[q],
                      ct01c_sb[:, q * 8:(q + 1) * 8], den_vb[q],
                      Sv[q], Su[q], ub[q], vb[q], ndf_sb[:, q:q + 1])

        def main_iter(boundaries=False):
            h = G // 2
            for q in range(h):
                u_half(q)
            if boundaries:
                tc.stage_boundary()
            for q in range(h, G):
                u_half(q)
            if boundaries:
                tc.stage_boundary()
            for q in range(h):
                v_half(q)
            if boundaries:
                tc.stage_boundary()
            for q in range(h, G):
                v_half(q)

        if n_main > 0:
            if static:
                for _ in range(n_main):
                    main_iter()
            elif staggered:
                with tc.For_i(0, n_main, 1, staggered_reset=True):
                    main_iter(boundaries=True)
            else:
                with tc.For_i(0, n_main, 1):
                    main_iter()

        for q in range(G):
            nc.vector.memset(v_lo[q][:], 0.0)
            nc.vector.tensor_copy(vf[q][:], v_col[q][:])
        for _ in range(n_tail):
            for q in range(G):
                half_tail(q, AT_hi, AT_lo, v_col[q], v_lo[q], uf[q], u_col[q],
                          u_lo[q], rt01c_sb[:, q * 8:(q + 1) * 8], den_uf[q],
                          Su[q], Sv[q], vb[q], ub[q], ntf_sb[:, q:q + 1])
            for q in range(G):
                half_tail(q, A_hi, A_lo, u_col[q], u_lo[q], vf[q], v_col[q],
                          v_lo[q], ct01c_sb[:, q * 8:(q + 1) * 8], den_vf[q],
                          Sv[q], Su[q], ub[q], vb[q], ndf_sb[:, q:q + 1])

        ep = P(tc.tile_pool(name="ep", bufs=2))
        for q in range(G):
            vrow = ep.tile([128, 256], F32, tag="vrow", name="vrow")
            ctr = ep.tile([128, 256], F32, tag="ctr", name="ctr")
            nc.sync.dma_start(ctr[:], ct01r[q])
            nc.vector.reciprocal(vrow[:], den_vf[q][:])
            nc.vector.tensor_mul(vrow[:], vrow[:], ctr[:])
            # border scalars to partition 0, then broadcast (gpsimd bcast only
            # works from partition 0 on HW): col 2g = v_b, 2g+1 = u_b
            sc8 = ep.tile([1, 8], F32, tag="sc8", name="sc8")
            for g in range(4):
                nc.vector.tensor_copy(sc8[:, 2 * g:2 * g + 1],
                                      vb[q][32 * g:32 * g + 1, 0:1])
                nc.vector.tensor_copy(sc8[:, 2 * g + 1:2 * g + 2],
                                      ub[q][32 * g:32 * g + 1, 0:1])
            scb = ep.tile([128, 8], F32, tag="scb", name="scb")
            nc.gpsimd.partition_broadcast(scb[:], sc8[:])
            for g in range(4):
                e = q * 4 + g
                vr0 = ep.tile([1, 256], F32, tag="vr0", name="vr0")
                nc.vector.tensor_copy(vr0[:], vrow[32 * g:32 * g + 1, :])
                vbc = ep.tile([128, 256], F32, tag="vbc", name="vbc")
                nc.gpsimd.partition_broadcast(vbc[:], vr0[:])
                Tt = ep.tile([128, 512], F32, tag="Tt", name="Tt")
                rm = ep.tile([128, 2], F32, tag="rm", name="rm")
                bc = ep.tile([128, 2], F32, tag="bc", name="bc")
                for t in range(2):
                    sl = slice(t * 256, (t + 1) * 256)
                    msl = slice(e * 512 + t * 256, e * 512 + (t + 1) * 256)
                    us = wslice(uf[q], g, t)
                    af = ep.tile([128, 256], F32, tag="afrec", name="afrec")
                    nc.vector.tensor_tensor(af[:], A_hi[:, msl], A_lo[:, msl], ALU.add)
                    nc.scalar.activation(Tt[:, sl], af[:], AF.Copy, bias=0.0, scale=us)
                    nc.vector.tensor_mul(Tt[:, sl], Tt[:, sl], vbc[:])
                    nc.vector.tensor_reduce(rm[:, t:t + 1], Tt[:, sl], axis=AX.X,
                                            op=ALU.max)
                    nc.vector.tensor_mul(bc[:, t:t + 1], us, scb[:, 2 * g:2 * g + 1])
                nc.vector.tensor_tensor(rm[:], rm[:], bc[:], ALU.max)
                # colmax (+ border row) broadcast across partitions
                cmb = ep.tile([128, 256], F32, tag="cmb", name="cmb")
                ar2 = ep.tile([128, 256], F32, tag="ar2", name="ar2")
                nc.gpsimd.partition_all_reduce(cmb[:], Tt[:, 0:256], channels=128,
                                               reduce_op=bass_isa.ReduceOp.max)
                nc.gpsimd.partition_all_reduce(ar2[:], Tt[:, 256:512], channels=128,
                                               reduce_op=bass_isa.ReduceOp.max)
                nc.vector.tensor_tensor(cmb[:], cmb[:], ar2[:], ALU.max)
                br = ep.tile([128, 256], F32, tag="br", name="br")
                nc.vector.tensor_scalar_mul(br[:], vbc[:], scb[:, 2 * g + 1:2 * g + 2])
                nc.vector.tensor_tensor(cmb[:], cmb[:], br[:], ALU.max)
                au8 = ep.tile([128, 512], U8, tag="au8", name="au8")
                eq = ep.tile([128, 512], F32, tag="eq", name="eq")
                for t in range(2):
                    sl = slice(t * 256, (t + 1) * 256)
                    nc.vector.tensor_scalar(eq[:, sl], Tt[:, sl], rm[:, t:t + 1],
                                            None, ALU.is_equal)
                    eq2 = ep.tile([128, 256], F32, tag="eq2", name="eq2")
                    nc.vector.tensor_tensor(eq2[:], Tt[:, sl], cmb[:], ALU.is_equal)
                    nc.vector.tensor_mul(eq[:, sl], eq[:, sl], eq2[:])
                    gt = ep.tile([128, 256], F32, tag="gt", name="gt")
                    nc.vector.tensor_scalar(gt[:], Tt[:, sl], 0.0, None, ALU.is_gt)
                    nc.vector.tensor_mul(eq[:, sl], eq[:, sl], gt[:])
                nc.vector.tensor_copy(au8[:], eq[:])
                nc.sync.dma_start(t_out[e], Tt[:])
                nc.sync.dma_start(a_out[e], au8[:])

    nc.compile()
    return nc


def _host_prep(aff_b, nd_b, nt_b):
    n = aff_b.shape[0]
    G = n // 4
    aff_dev = np.ascontiguousarray(
        aff_b.reshape(n, 2, 128, 256).transpose(0, 2, 1, 3).reshape(n, 128, 512))
    afft = np.ascontiguousarray(aff_b.transpose(0, 2, 1))
    afft_dev = np.ascontiguousarray(
        afft.reshape(n, 2, 128, 256).transpose(0, 2, 1, 3).reshape(n, 128, 512))
    p = np.arange(128)
    k = np.arange(2)[:, None] * 128 + p[None, :]
    rt01c = np.zeros((G, 128, 8), np.float32)
    ct01c = np.zeros((G, 128, 8), np.float32)
    ntf = np.zeros((G, 128, 1), np.float32)
    ndf = np.zeros((G, 128, 1), np.float32)
    ct01r = np.zeros((G, 128, 256), np.float32)
    for q in range(G):
        for g in range(4):
            e = q * 4 + g
            for tt in range(2):
                rt01c[q, :, tt * 4 + g] = (k[tt] < nt_b[e])
                ct01c[q, :, tt * 4 + g] = (k[tt] < nd_b[e])
            ntf[q, 32 * g:32 * g + 32, 0] = np.float32(nt_b[e])
            ndf[q, 32 * g:32 * g + 32, 0] = np.float32(nd_b[e])
            ct01r[q, 32 * g, :] = (np.arange(256) < nd_b[e])
    return {
        "aff": aff_dev, "afft": afft_dev, "rt01c": rt01c, "ct01c": ct01c,
        "ntf": ntf, "ndf": ndf, "ct01r": ct01r,
        "ones_in": np.ones((128, 1), np.float32),
    }


def _host_post(t_dev, a_dev, nd_b, nt_b):
    n = t_dev.shape[0]
    T = t_dev.reshape(n, 128, 2, 256).transpose(0, 2, 1, 3).reshape(n, 256, 256)
    A_ = a_dev.reshape(n, 128, 2, 256).transpose(0, 2, 1, 3).reshape(n, 256, 256)
    t_flat = np.zeros((n, MT * MD), np.float32)
    a_flat = np.zeros((n, MT * MD), bool)
    for b in range(n):
        nt, nd = int(nt_b[b]), int(nd_b[b])
        t_flat[b, :nt * nd] = T[b, :nt, :nd].ravel()
        a_flat[b, :nt * nd] = A_[b, :nt, :nd].astype(bool).ravel()
    return t_flat, a_flat


_NC_CACHE = {}


def _get_nc():
    key = (N_EX, N_ITERS, N_TAIL)
    if key not in _NC_CACHE:
        _NC_CACHE[key] = _build(*key)
    return _NC_CACHE[key]


def kernel(affinity_scores, num_detections, num_tracklets):
    aff = np.asarray(affinity_scores, dtype=np.float32)
    nd = np.asarray(num_detections, dtype=np.int32)
    nt = np.asarray(num_tracklets, dtype=np.int32)
    assert aff.shape == (B, MT, MD), aff.shape

    nc = _get_nc()
    in_maps = []
    for c in range(N_CORES):
        s = slice(c * N_EX, (c + 1) * N_EX)
        in_maps.append(_host_prep(aff[s], nd[s], nt[s]))
    res = bass_utils.run_bass_kernel_spmd(nc, in_maps, core_ids=list(range(N_CORES)))

    t_full = np.zeros((B, MT * MD), np.float32)
    a_full = np.zeros((B, MT * MD), bool)
    for c in range(N_CORES):
        s = slice(c * N_EX, (c + 1) * N_EX)
        t_flat, a_flat = _host_post(res.results[c]["t_out"], res.results[c]["a_out"],
                                    nd[s], nt[s])
        t_full[s] = t_flat
        a_full[s] = a_flat
    return t_full, a_full


# revision 11
# speedup vs baseline: 1693.7076x; 1693.7076x over previous
"""Trainium2 Bass kernel for nn_AssociationLayer (batched masked Sinkhorn).

Self-contained: kernel(**inputs) takes the FULL unsharded inputs
(affinity_scores [256,256,256] f32, num_detections [256] i32,
num_tracklets [256] i32) and returns (sinkhorn_dense [256, 65536] f32,
assignment_dense [256, 65536] bool), matching the reference.

Distribution: pure data parallelism — batch is sharded 8 x 32 across the 8
NeuronCores; each core solves its 32 examples independently (no collectives).

Per-core algorithm (groups of 4 examples):
  A = exp(10*aff), stored as A_hi + A_lo (two bf16 tiles, ~2^-17 precision)
  95 main iterations: bf16 matvecs on TensorE (4-way column-packed via
    tile_position), denominators moved PSUM->SBUF by ScalarE (+border bias),
    transposed back to weight layout by TensorE, reciprocal+mask on VectorE
  5 tail iterations: 3-term split-bf16 matvecs (f32-grade precision)
  epilogue: transport T = (u_i*A_ij)*v_j, mutual row/col argmax assignment
  Ragged pack to the reference's flattened layout happens on host.
"""
import numpy as np
from contextlib import ExitStack

import concourse.bass as bass
import concourse.bacc as bacc
import concourse.mybir as mybir
from concourse import tile
from concourse import bass_utils
from concourse.masks import make_identity
import concourse.bass_isa as bass_isa

F32 = mybir.dt.float32
BF16 = mybir.dt.bfloat16
U8 = mybir.dt.uint8
AF = mybir.ActivationFunctionType
ALU = mybir.AluOpType
AX = mybir.AxisListType

N_CORES = 8
B, MT, MD = 256, 256, 256
N_EX = B // N_CORES              # 32 examples per core
N_ITERS = 100
N_TAIL = 5


def _build(n_ex=N_EX, n_iters=N_ITERS, n_tail=N_TAIL, static=False, staggered=False, ablate=(), bufs=(3, 3, 2)):
    assert n_ex % 4 == 0
    G = n_ex // 4
    n_main = n_iters - n_tail
    nc = bacc.Bacc("TRN2", target_bir_lowering=False, debug=False)

    aff = nc.dram_tensor("aff", [n_ex, 128, 512], F32, kind="ExternalInput").ap()
    afft = nc.dram_tensor("afft", [n_ex, 128, 512], F32, kind="ExternalInput").ap()
    rt01c = nc.dram_tensor("rt01c", [G, 128, 8], F32, kind="ExternalInput").ap()
    ct01c = nc.dram_tensor("ct01c", [G, 128, 8], F32, kind="ExternalInput").ap()
    ntf = nc.dram_tensor("ntf", [G, 128, 1], F32, kind="ExternalInput").ap()
    ndf = nc.dram_tensor("ndf", [G, 128, 1], F32, kind="ExternalInput").ap()
    ct01r = nc.dram_tensor("ct01r", [G, 128, 256], F32, kind="ExternalInput").ap()
    ones_in = nc.dram_tensor("ones_in", [128, 1], F32, kind="ExternalInput").ap()
    t_out = nc.dram_tensor("t_out", [n_ex, 128, 512], F32, kind="ExternalOutput").ap()
    a_out = nc.dram_tensor("a_out", [n_ex, 128, 512], U8, kind="ExternalOutput").ap()

    with tile.TileContext(nc) as tc, ExitStack() as ctx:
        P = ctx.enter_context
        const = P(tc.tile_pool(name="const", bufs=1))
        mats = P(tc.tile_pool(name="mats", bufs=1))
        state = P(tc.tile_pool(name="state", bufs=1))
        pp_mv = P(tc.tile_pool(name="pp_mv", bufs=1, space="PSUM"))
        pp_tp = P(tc.tile_pool(name="pp_tp", bufs=1, space="PSUM"))
        pp_sm = P(tc.tile_pool(name="pp_sm", bufs=1, space="PSUM"))

        ident_f = const.tile([128, 128], F32, tag="ident_f")
        ident_b = const.tile([128, 128], BF16, tag="ident_b")
        make_identity(nc, ident_f[:])
        nc.vector.tensor_copy(ident_b[:], ident_f[:])
        ones_f = const.tile([128, 1], F32, tag="ones_f")
        ones_b = const.tile([128, 1], BF16, tag="ones_b")
        nc.sync.dma_start(ones_f[:], ones_in[:])
        nc.vector.tensor_copy(ones_b[:], ones_f[:])

        rt01c_sb = const.tile([128, G * 8], F32, tag="rt01c")
        ct01c_sb = const.tile([128, G * 8], F32, tag="ct01c")
        ntf_sb = const.tile([128, G], F32, tag="ntf")
        ndf_sb = const.tile([128, G], F32, tag="ndf")
        for q in range(G):
            nc.sync.dma_start(rt01c_sb[:, q * 8:(q + 1) * 8], rt01c[q])
            nc.sync.dma_start(ct01c_sb[:, q * 8:(q + 1) * 8], ct01c[q])
            nc.sync.dma_start(ntf_sb[:, q:q + 1], ntf[q])
            nc.sync.dma_start(ndf_sb[:, q:q + 1], ndf[q])

        A_hi = mats.tile([128, n_ex * 512], BF16, tag="A_hi")
        A_lo = mats.tile([128, n_ex * 512], BF16, tag="A_lo")
        AT_hi = mats.tile([128, n_ex * 512], BF16, tag="AT_hi")
        AT_lo = mats.tile([128, n_ex * 512], BF16, tag="AT_lo")
        with tc.tile_pool(name="stage", bufs=3) as stg:
            for e in range(n_ex):
                for src, hi, lo in ((aff, A_hi, A_lo), (afft, AT_hi, AT_lo)):
                    sl = slice(e * 512, (e + 1) * 512)
                    st = stg.tile([128, 512], F32, tag="ldstage", name="ldstage")
                    nc.sync.dma_start(st[:], src[e])
                    af = stg.tile([128, 512], F32, tag="af32", name="af32")
                    nc.scalar.activation(af[:], st[:], AF.Exp, bias=0.0, scale=10.0)
                    nc.scalar.activation(hi[:, sl], af[:], AF.Copy, bias=0.0, scale=1.0)
                    nc.vector.tensor_tensor(lo[:, sl], af[:], hi[:, sl], ALU.subtract)

        def tiles(shape, dt_, pfx):
            return [state.tile(shape, dt_, tag=f"{pfx}{q}", name=f"{pfx}{q}")
                    for q in range(G)]
        u_col = tiles([128, 8], BF16, "u")
        v_col = tiles([128, 8], BF16, "v")
        uf = tiles([128, 8], F32, "uf")
        vf = tiles([128, 8], F32, "vf")
        u_lo = tiles([128, 8], BF16, "ulo")
        v_lo = tiles([128, 8], BF16, "vlo")
        den_ub = tiles([128, 256], BF16, "dub")
        den_vb = tiles([128, 256], BF16, "dvb")
        den_uf = tiles([128, 256], F32, "duf")
        den_vf = tiles([128, 256], F32, "dvf")
        Su = tiles([128, 1], F32, "Su")
        Sv = tiles([128, 1], F32, "Sv")
        ub = tiles([128, 1], F32, "ub")
        vb = tiles([128, 1], F32, "vb")
        rec8 = tiles([128, 8], F32, "r8")
        dbt = tiles([128, 1], F32, "db")

        NPM, NTP, NSM = bufs
        pmv = [pp_mv.tile([128, 256], F32, tag=f"pmv{i}", name=f"pmv{i}")
               for i in range(NPM)]
        ptp = [pp_tp.tile([128, 256], F32, tag=f"ptp{i}", name=f"ptp{i}")
               for i in range(NTP)]
        psm = [pp_sm.tile([128, 2], F32, tag=f"psm{i}", name=f"psm{i}")
               for i in range(NSM)]
        for t_ in pmv + ptp + psm:
            nc.vector.memset(t_[:, :], 0.0)

        for q in range(G):
            nc.vector.tensor_copy(v_col[q][:], ct01c_sb[:, q * 8:(q + 1) * 8])
            nc.vector.memset(vb[q][:], 1.0)
            nc.vector.tensor_copy(Sv[q][:], ndf_sb[:, q:q + 1])

        def wslice(col, g, t):
            return col[:, :].rearrange("p (t g) -> p g t", g=4)[:, g, t:t + 1]

        def sslice(col, g):
            return col[:, :].rearrange("p (t g) -> p g t", g=4)[:, g, :]

        def border(q, S_cur, b_cur, b_new, nf_sb):
            nc.vector.tensor_add(dbt[q][:], S_cur[:], b_cur[:])
            nc.vector.reciprocal(dbt[q][:], dbt[q][:])
            nc.vector.tensor_mul(b_new[:], nf_sb, dbt[q][:])

        def half_bf16_A(q, mat, w_col, mask_sb, den_sb, S_new, S_cur,
                        b_cur, b_new, nf_sb):
            pm = pmv[q % NPM]
            for g in range(4):
                e = q * 4 + g
                for t in range(2):
                    nc.tensor.matmul(
                        pm[32 * g:32 * g + 1, :], wslice(w_col, g, t),
                        mat[:, e * 512 + t * 256: e * 512 + (t + 1) * 256],
                        start=(t == 0), stop=(t == 1),
                        tile_position=(0, 32 * g), skip_group_check=True)
            if "border" not in ablate:
                border(q, S_cur, b_cur, b_new, nf_sb)
            nc.scalar.activation(den_sb[:], pm[:, :], AF.Identity,
                                 bias=b_cur[:, 0:1], scale=1.0)

        def half_bf16_B(q, out_col, mask_sb, den_sb, S_new):
            pt, ps = ptp[q % NTP], psm[q % NSM]
            ptb = pt[:, :].bitcast(BF16)
            if "tr" not in ablate:
                nc.tensor.transpose(ptb[:, 0:128], den_sb[:, 0:128], ident_b[:])
                nc.tensor.transpose(ptb[:, 128:256], den_sb[:, 128:256], ident_b[:])
            tp_src = ptb[:, 0:256].rearrange("p (t g r) -> p t g r", t=2, g=4)[:, :, :, 0]
            nc.vector.reciprocal(rec8[q][:].rearrange("p (t g) -> p t g", t=2), tp_src)
            nc.vector.tensor_mul(out_col[:], rec8[q][:], mask_sb)
            if "sums" not in ablate:
                for g in range(4):
                    nc.tensor.matmul(ps[32 * g:32 * g + 1, :], ones_b[:],
                                     sslice(out_col, g), start=True, stop=True,
                                     tile_position=(0, 32 * g), skip_group_check=True)
                nc.vector.tensor_reduce(S_new[:], ps[:, :], axis=AX.X, op=ALU.add)

        def half_bf16(q, mat, w_col, out_col, mask_sb, den_sb, S_new, S_cur,
                      b_cur, b_new, nf_sb):
            half_bf16_A(q, mat, w_col, mask_sb, den_sb, S_new, S_cur,
                        b_cur, b_new, nf_sb)
            half_bf16_B(q, out_col, mask_sb, den_sb, S_new)

        def half_tail(q, mat_hi, mat_lo, whi, wlo, out_f, out_hi, out_lo,
                      mask_sb, den_sb, S_new, S_cur, b_cur, b_new, nf_sb):
            pm, pt, ps = pmv[q % NPM], ptp[q % NTP], psm[q % NSM]
            for g in range(4):
                e = q * 4 + g
                for t in range(2):
                    msl = slice(e * 512 + t * 256, e * 512 + (t + 1) * 256)
                    nc.tensor.matmul(pm[32 * g:32 * g + 1, :], wslice(whi, g, t),
                                     mat_hi[:, msl], start=(t == 0), stop=False,
                                     tile_position=(0, 32 * g), skip_group_check=True)
                    nc.tensor.matmul(pm[32 * g:32 * g + 1, :], wslice(wlo, g, t),
                                     mat_hi[:, msl], start=False, stop=False,
                                     tile_position=(0, 32 * g), skip_group_check=True)
                    nc.tensor.matmul(pm[32 * g:32 * g + 1, :], wslice(whi, g, t),
                                     mat_lo[:, msl], start=False, stop=(t == 1),
                                     tile_position=(0, 32 * g), skip_group_check=True)
            border(q, S_cur, b_cur, b_new, nf_sb)
            nc.scalar.activation(den_sb[:], pm[:, :], AF.Identity,
                                 bias=b_cur[:, 0:1], scale=1.0)
            nc.tensor.transpose(pt[:, 0:128], den_sb[:, 0:128], ident_f[:])
            nc.tensor.transpose(pt[:, 128:256], den_sb[:, 128:256], ident_f[:])
            tp_src = pt[:, 0:256].rearrange("p (t g r) -> p t g r", t=2, g=4)[:, :, :, 0]
            nc.vector.reciprocal(rec8[q][:].rearrange("p (t g) -> p t g", t=2), tp_src)
            nc.vector.tensor_mul(out_f[:], rec8[q][:], mask_sb)
            nc.scalar.activation(out_hi[:], out_f[:], AF.Copy, bias=0.0, scale=1.0)
            nc.vector.tensor_tensor(out_lo[:], out_f[:], out_hi[:], ALU.subtract)
            for g in range(4):
                nc.tensor.matmul(ps[32 * g:32 * g + 1, :], ones_f[:],
                                 sslice(out_f, g), start=True, stop=True,
                                 tile_position=(0, 32 * g), skip_group_check=True)
            nc.vector.tensor_reduce(S_new[:], ps[:, :], axis=AX.X, op=ALU.add)

        def u_args(q):
            return (q, AT_hi, v_col[q], u_col[q],
                    rt01c_sb[:, q * 8:(q + 1) * 8], den_ub[q],
                    Su[q], Sv[q], vb[q], ub[q], ntf_sb[:, q:q + 1])

        def v_args(q):
            return (q, A_hi, u_col[q], v_col[q],
                    ct01c_sb[:, q * 8:(q + 1) * 8], den_vb[q],
                    Sv[q], Su[q], ub[q], vb[q], ndf_sb[:, q:q + 1])

        def u_half(q):
            half_bf16(*u_args(q))

        def v_half(q):
            half_bf16(*v_args(q))

        def stage_A(args):
            (q, mat, w_col, out_col, mask_sb, den_sb, S_new, S_cur,
             b_cur, b_new, nf_sb) = args
            half_bf16_A(q, mat, w_col, mask_sb, den_sb, S_new, S_cur,
                        b_cur, b_new, nf_sb)

        def stage_B(args):
            (q, mat, w_col, out_col, mask_sb, den_sb, S_new, S_cur,
             b_cur, b_new, nf_sb) = args
            half_bf16_B(q, out_col, mask_sb, den_sb, S_new)

        def main_iter(boundaries=False, skewed=True):
            if not skewed:
                h = G // 2
                for q in range(h):
                    u_half(q)
                if boundaries:
                    tc.stage_boundary()
                for q in range(h, G):
                    u_half(q)
                if boundaries:
                    tc.stage_boundary()
                for q in range(h):
                    v_half(q)
                if boundaries:
                    tc.stage_boundary()
                for q in range(h, G):
                    v_half(q)
                return
            # software-pipelined emission: stage A of half k+1 lands on the PE
            # queue before stage B of half k, hiding the ACT->transpose stall
            halves = [u_args(q) for q in range(G)] + [v_args(q) for q in range(G)]
            SKEW = 1
            for k in range(2 * G + SKEW):
                if k < 2 * G:
                    stage_A(halves[k])
                if k >= SKEW:
                    stage_B(halves[k - SKEW])

        if n_main > 0:
            if static:
                for _ in range(n_main):
                    main_iter()
            elif staggered:
                with tc.For_i(0, n_main, 1, staggered_reset=True):
                    main_iter(boundaries=True)
            else:
                with tc.For_i(0, n_main, 1):
                    main_iter()

        for q in range(G):
            nc.vector.memset(v_lo[q][:], 0.0)
            nc.vector.tensor_copy(vf[q][:], v_col[q][:])
        for _ in range(n_tail):
            for q in range(G):
                half_tail(q, AT_hi, AT_lo, v_col[q], v_lo[q], uf[q], u_col[q],
                          u_lo[q], rt01c_sb[:, q * 8:(q + 1) * 8], den_uf[q],
                          Su[q], Sv[q], vb[q], ub[q], ntf_sb[:, q:q + 1])
            for q in range(G):
                half_tail(q, A_hi, A_lo, u_col[q], u_lo[q], vf[q], v_col[q],
                          v_lo[q], ct01c_sb[:, q * 8:(q + 1) * 8], den_vf[q],
                          Sv[q], Su[q], ub[q], vb[q], ndf_sb[:, q:q + 1])

        ep = P(tc.tile_pool(name="ep", bufs=2))
        for q in range(G):
            vrow = ep.tile([128, 256], F32, tag="vrow", name="vrow")
            ctr = ep.tile([128, 256], F32, tag="ctr", name="ctr")
            nc.sync.dma_start(ctr[:], ct01r[q])
            nc.vector.reciprocal(vrow[:], den_vf[q][:])
            nc.vector.tensor_mul(vrow[:], vrow[:], ctr[:])
            # border scalars to partition 0, then broadcast (gpsimd bcast only
            # works from partition 0 on HW): col 2g = v_b, 2g+1 = u_b
            sc8 = ep.tile([1, 8], F32, tag="sc8", name="sc8")
            for g in range(4):
                nc.vector.tensor_copy(sc8[:, 2 * g:2 * g + 1],
                                      vb[q][32 * g:32 * g + 1, 0:1])
                nc.vector.tensor_copy(sc8[:, 2 * g + 1:2 * g + 2],
                                      ub[q][32 * g:32 * g + 1, 0:1])
            scb = ep.tile([128, 8], F32, tag="scb", name="scb")
            nc.gpsimd.partition_broadcast(scb[:], sc8[:])
            for g in range(4):
                e = q * 4 + g
                vr0 = ep.tile([1, 256], F32, tag="vr0", name="vr0")
                nc.vector.tensor_copy(vr0[:], vrow[32 * g:32 * g + 1, :])
                vbc = ep.tile([128, 256], F32, tag="vbc", name="vbc")
                nc.gpsimd.partition_broadcast(vbc[:], vr0[:])
                Tt = ep.tile([128, 512], F32, tag="Tt", name="Tt")
                rm = ep.tile([128, 2], F32, tag="rm", name="rm")
                bc = ep.tile([128, 2], F32, tag="bc", name="bc")
                for t in range(2):
                    sl = slice(t * 256, (t + 1) * 256)
                    msl = slice(e * 512 + t * 256, e * 512 + (t + 1) * 256)
                    us = wslice(uf[q], g, t)
                    af = ep.tile([128, 256], F32, tag="afrec", name="afrec")
                    nc.vector.tensor_tensor(af[:], A_hi[:, msl], A_lo[:, msl], ALU.add)
                    nc.scalar.activation(Tt[:, sl], af[:], AF.Copy, bias=0.0, scale=us)
                    nc.vector.tensor_mul(Tt[:, sl], Tt[:, sl], vbc[:])
                    nc.vector.tensor_reduce(rm[:, t:t + 1], Tt[:, sl], axis=AX.X,
                                            op=ALU.max)
                    nc.vector.tensor_mul(bc[:, t:t + 1], us, scb[:, 2 * g:2 * g + 1])
                nc.vector.tensor_tensor(rm[:], rm[:], bc[:], ALU.max)
                # colmax (+ border row) broadcast across partitions
                cmb = ep.tile([128, 256], F32, tag="cmb", name="cmb")
                ar2 = ep.tile([128, 256], F32, tag="ar2", name="ar2")
                nc.gpsimd.partition_all_reduce(cmb[:], Tt[:, 0:256], channels=128,
                                               reduce_op=bass_isa.ReduceOp.max)
                nc.gpsimd.partition_all_reduce(ar2[:], Tt[:, 256:512], channels=128,
                                               reduce_op=bass_isa.ReduceOp.max)
                nc.vector.tensor_tensor(cmb[:], cmb[:], ar2[:], ALU.max)
                br = ep.tile([128, 256], F32, tag="br", name="br")
                nc.vector.tensor_scalar_mul(br[:], vbc[:], scb[:, 2 * g + 1:2 * g + 2])
                nc.vector.tensor_tensor(cmb[:], cmb[:], br[:], ALU.max)
                au8 = ep.tile([128, 512], U8, tag="au8", name="au8")
                eq = ep.tile([128, 512], F32, tag="eq", name="eq")
                for t in range(2):
                    sl = slice(t * 256, (t + 1) * 256)
                    nc.vector.tensor_scalar(eq[:, sl], Tt[:, sl], rm[:, t:t + 1],
                                            None, ALU.is_equal)
                    eq2 = ep.tile([128, 256], F32, tag="eq2", name="eq2")
                    nc.vector.tensor_tensor(eq2[:], Tt[:, sl], cmb[:], ALU.is_equal)
                    nc.vector.tensor_mul(eq[:, sl], eq[:, sl], eq2[:])
                    gt = ep.tile([128, 256], F32, tag="gt", name="gt")
                    nc.vector.tensor_scalar(gt[:], Tt[:, sl], 0.0, None, ALU.is_gt)
                    nc.vector.tensor_mul(eq[:, sl], eq[:, sl], gt[:])
                nc.vector.tensor_copy(au8[:], eq[:])
                nc.sync.dma_start(t_out[e], Tt[:])
                nc.sync.dma_start(a_out[e], au8[:])

    nc.compile()
    return nc


def _host_prep(aff_b, nd_b, nt_b):
    n = aff_b.shape[0]
    G = n // 4
    aff_dev = np.ascontiguousarray(
        aff_b.reshape(n, 2, 128, 256).transpose(0, 2, 1, 3).reshape(n, 128, 512))
    afft = np.ascontiguousarray(aff_b.transpose(0, 2, 1))
    afft_dev = np.ascontiguousarray(
        afft.reshape(n, 2, 128, 256).transpose(0, 2, 1, 3).reshape(n, 128, 512))
    p = np.arange(128)
    k = np.arange(2)[:, None] * 128 + p[None, :]
    rt01c = np.zeros((G, 128, 8), np.float32)
    ct01c = np.zeros((G, 128, 8), np.float32)
    ntf = np.zeros((G, 128, 1), np.float32)
    ndf = np.zeros((G, 128, 1), np.float32)
    ct01r = np.zeros((G, 128, 256), np.float32)
    for q in range(G):
        for g in range(4):
            e = q * 4 + g
            for tt in range(2):
                rt01c[q, :, tt * 4 + g] = (k[tt] < nt_b[e])
                ct01c[q, :, tt * 4 + g] = (k[tt] < nd_b[e])
            ntf[q, 32 * g:32 * g + 32, 0] = np.float32(nt_b[e])
            ndf[q, 32 * g:32 * g + 32, 0] = np.float32(nd_b[e])
            ct01r[q, 32 * g, :] = (np.arange(256) < nd_b[e])
    return {
        "aff": aff_dev, "afft": afft_dev, "rt01c": rt01c, "ct01c": ct01c,
        "ntf": ntf, "ndf": ndf, "ct01r": ct01r,
        "ones_in": np.ones((128, 1), np.float32),
    }


def _host_post(t_dev, a_dev, nd_b, nt_b):
    n = t_dev.shape[0]
    T = t_dev.reshape(n, 128, 2, 256).transpose(0, 2, 1, 3).reshape(n, 256, 256)
    A_ = a_dev.reshape(n, 128, 2, 256).transpose(0, 2, 1, 3).reshape(n, 256, 256)
    t_flat = np.zeros((n, MT * MD), np.float32)
    a_flat = np.zeros((n, MT * MD), bool)
    for b in range(n):
        nt, nd = int(nt_b[b]), int(nd_b[b])
        t_flat[b, :nt * nd] = T[b, :nt, :nd].ravel()
        a_flat[b, :nt * nd] = A_[b, :nt, :nd].astype(bool).ravel()
    return t_flat, a_flat


_NC_CACHE = {}


def _get_nc():
    key = (N_EX, N_ITERS, N_TAIL)
    if key not in _NC_CACHE:
        _NC_CACHE[key] = _build(*key)
    return _NC_CACHE[key]


def kernel(affinity_scores, num_detections, num_tracklets):
    aff = np.asarray(affinity_scores, dtype=np.float32)
    nd = np.asarray(num_detections, dtype=np.int32)
    nt = np.asarray(num_tracklets, dtype=np.int32)
    assert aff.shape == (B, MT, MD), aff.shape

    nc = _get_nc()
    in_maps = []
    for c in range(N_CORES):
        s = slice(c * N_EX, (c + 1) * N_EX)
        in_maps.append(_host_prep(aff[s], nd[s], nt[s]))
    res = bass_utils.run_bass_kernel_spmd(nc, in_maps, core_ids=list(range(N_CORES)))

    t_full = np.zeros((B, MT * MD), np.float32)
    a_full = np.zeros((B, MT * MD), bool)
    for c in range(N_CORES):
        s = slice(c * N_EX, (c + 1) * N_EX)
        t_flat, a_flat = _host_post(res.results[c]["t_out"], res.results[c]["a_out"],
                                    nd[s], nt[s])
        t_full[s] = t_flat
        a_full[s] = a_flat
    return t_full, a_full


# revision 12
# speedup vs baseline: 1717.3745x; 1.0140x over previous
"""Trainium2 Bass kernel for nn_AssociationLayer (batched masked Sinkhorn).

Self-contained: kernel(**inputs) takes the FULL unsharded inputs
(affinity_scores [256,256,256] f32, num_detections [256] i32,
num_tracklets [256] i32) and returns (sinkhorn_dense [256, 65536] f32,
assignment_dense [256, 65536] bool), matching the reference.

Distribution: pure data parallelism — batch is sharded 8 x 32 across the 8
NeuronCores; each core solves its 32 examples independently (no collectives).

Per-core algorithm (groups of 4 examples):
  A = exp(10*aff), stored as A_hi + A_lo (two bf16 tiles, ~2^-17 precision)
  95 main iterations: bf16 matvecs on TensorE (4-way column-packed via
    tile_position), denominators moved PSUM->SBUF by ScalarE (+border bias),
    transposed back to weight layout by TensorE, reciprocal+mask on VectorE
  5 tail iterations: 3-term split-bf16 matvecs (f32-grade precision)
  epilogue: transport T = (u_i*A_ij)*v_j, mutual row/col argmax assignment
  Ragged pack to the reference's flattened layout happens on host.
"""
import numpy as np
from contextlib import ExitStack

import concourse.bass as bass
import concourse.bacc as bacc
import concourse.mybir as mybir
from concourse import tile
from concourse import bass_utils
from concourse.masks import make_identity
import concourse.bass_isa as bass_isa

F32 = mybir.dt.float32
BF16 = mybir.dt.bfloat16
U8 = mybir.dt.uint8
AF = mybir.ActivationFunctionType
ALU = mybir.AluOpType
AX = mybir.AxisListType

N_CORES = 8
B, MT, MD = 256, 256, 256
N_EX = B // N_CORES              # 32 examples per core
N_ITERS = 100
N_TAIL = 5


def _build(n_ex=N_EX, n_iters=N_ITERS, n_tail=N_TAIL, static=False, staggered=False, ablate=(), bufs=(3, 3, 2)):
    assert n_ex % 4 == 0
    G = n_ex // 4
    n_main = n_iters - n_tail
    nc = bacc.Bacc("TRN2", target_bir_lowering=False, debug=False)

    aff = nc.dram_tensor("aff", [n_ex, 128, 512], F32, kind="ExternalInput").ap()
    afft = nc.dram_tensor("afft", [n_ex, 128, 512], F32, kind="ExternalInput").ap()
    rt01c = nc.dram_tensor("rt01c", [G, 128, 8], F32, kind="ExternalInput").ap()
    ct01c = nc.dram_tensor("ct01c", [G, 128, 8], F32, kind="ExternalInput").ap()
    ntf = nc.dram_tensor("ntf", [G, 128, 1], F32, kind="ExternalInput").ap()
    ndf = nc.dram_tensor("ndf", [G, 128, 1], F32, kind="ExternalInput").ap()
    ct01r = nc.dram_tensor("ct01r", [G, 128, 256], F32, kind="ExternalInput").ap()
    ones_in = nc.dram_tensor("ones_in", [128, 1], F32, kind="ExternalInput").ap()
    t_out = nc.dram_tensor("t_out", [n_ex, 128, 512], F32, kind="ExternalOutput").ap()
    a_out = nc.dram_tensor("a_out", [n_ex, 128, 512], U8, kind="ExternalOutput").ap()

    with tile.TileContext(nc) as tc, ExitStack() as ctx:
        P = ctx.enter_context
        const = P(tc.tile_pool(name="const", bufs=1))
        mats = P(tc.tile_pool(name="mats", bufs=1))
        state = P(tc.tile_pool(name="state", bufs=1))
        pp_mv = P(tc.tile_pool(name="pp_mv", bufs=1, space="PSUM"))
        pp_tp = P(tc.tile_pool(name="pp_tp", bufs=1, space="PSUM"))
        pp_sm = P(tc.tile_pool(name="pp_sm", bufs=1, space="PSUM"))

        ident_f = const.tile([128, 128], F32, tag="ident_f")
        ident_b = const.tile([128, 128], BF16, tag="ident_b")
        make_identity(nc, ident_f[:])
        nc.vector.tensor_copy(ident_b[:], ident_f[:])
        ones_f = const.tile([128, 1], F32, tag="ones_f")
        ones_b = const.tile([128, 1], BF16, tag="ones_b")
        nc.sync.dma_start(ones_f[:], ones_in[:])
        nc.vector.tensor_copy(ones_b[:], ones_f[:])

        rt01c_sb = const.tile([128, G * 8], F32, tag="rt01c")
        ct01c_sb = const.tile([128, G * 8], F32, tag="ct01c")
        ntf_sb = const.tile([128, G], F32, tag="ntf")
        ndf_sb = const.tile([128, G], F32, tag="ndf")
        for q in range(G):
            nc.sync.dma_start(rt01c_sb[:, q * 8:(q + 1) * 8], rt01c[q])
            nc.sync.dma_start(ct01c_sb[:, q * 8:(q + 1) * 8], ct01c[q])
            nc.sync.dma_start(ntf_sb[:, q:q + 1], ntf[q])
            nc.sync.dma_start(ndf_sb[:, q:q + 1], ndf[q])

        A_hi = mats.tile([128, n_ex * 512], BF16, tag="A_hi")
        A_lo = mats.tile([128, n_ex * 512], BF16, tag="A_lo")
        AT_hi = mats.tile([128, n_ex * 512], BF16, tag="AT_hi")
        AT_lo = mats.tile([128, n_ex * 512], BF16, tag="AT_lo")
        with tc.tile_pool(name="stage", bufs=3) as stg:
            for e in range(n_ex):
                for src, hi, lo in ((aff, A_hi, A_lo), (afft, AT_hi, AT_lo)):
                    sl = slice(e * 512, (e + 1) * 512)
                    st = stg.tile([128, 512], F32, tag="ldstage", name="ldstage")
                    nc.sync.dma_start(st[:], src[e])
                    af = stg.tile([128, 512], F32, tag="af32", name="af32")
                    nc.scalar.activation(af[:], st[:], AF.Exp, bias=0.0, scale=10.0)
                    nc.scalar.activation(hi[:, sl], af[:], AF.Copy, bias=0.0, scale=1.0)
                    nc.vector.tensor_tensor(lo[:, sl], af[:], hi[:, sl], ALU.subtract)

        def tiles(shape, dt_, pfx):
            return [state.tile(shape, dt_, tag=f"{pfx}{q}", name=f"{pfx}{q}")
                    for q in range(G)]
        u_col = tiles([128, 8], BF16, "u")
        v_col = tiles([128, 8], BF16, "v")
        uf = tiles([128, 8], F32, "uf")
        vf = tiles([128, 8], F32, "vf")
        u_lo = tiles([128, 8], BF16, "ulo")
        v_lo = tiles([128, 8], BF16, "vlo")
        den_ub = tiles([128, 256], BF16, "dub")
        den_vb = tiles([128, 256], BF16, "dvb")
        den_uf = tiles([128, 256], F32, "duf")
        den_vf = tiles([128, 256], F32, "dvf")
        Su = tiles([128, 1], F32, "Su")
        Sv = tiles([128, 1], F32, "Sv")
        ub = tiles([128, 1], F32, "ub")
        vb = tiles([128, 1], F32, "vb")
        rec8 = tiles([128, 8], F32, "r8")
        dbt = tiles([128, 1], F32, "db")

        NPM, NTP, NSM = bufs
        pmv = [pp_mv.tile([128, 256], F32, tag=f"pmv{i}", name=f"pmv{i}")
               for i in range(NPM)]
        ptp = [pp_tp.tile([128, 256], F32, tag=f"ptp{i}", name=f"ptp{i}")
               for i in range(NTP)]
        psm = [pp_sm.tile([128, 2], F32, tag=f"psm{i}", name=f"psm{i}")
               for i in range(NSM)]
        for t_ in pmv + ptp + psm:
            nc.vector.memset(t_[:, :], 0.0)

        for q in range(G):
            nc.vector.tensor_copy(v_col[q][:], ct01c_sb[:, q * 8:(q + 1) * 8])
            nc.vector.memset(vb[q][:], 1.0)
            nc.vector.tensor_copy(Sv[q][:], ndf_sb[:, q:q + 1])

        def wslice(col, g, t):
            return col[:, :].rearrange("p (t g) -> p g t", g=4)[:, g, t:t + 1]

        def sslice(col, g):
            return col[:, :].rearrange("p (t g) -> p g t", g=4)[:, g, :]

        def border(q, S_cur, b_cur, b_new, nf_sb):
            nc.vector.tensor_add(dbt[q][:], S_cur[:], b_cur[:])
            nc.vector.reciprocal(dbt[q][:], dbt[q][:])
            nc.vector.tensor_mul(b_new[:], nf_sb, dbt[q][:])

        def half_bf16_A(q, mat, w_col, mask_sb, den_sb, S_new, S_cur,
                        b_cur, b_new, nf_sb):
            pm = pmv[q % NPM]
            for g in range(4):
                e = q * 4 + g
                for t in range(2):
                    nc.tensor.matmul(
                        pm[32 * g:32 * g + 1, :], wslice(w_col, g, t),
                        mat[:, e * 512 + t * 256: e * 512 + (t + 1) * 256],
                        start=(t == 0), stop=(t == 1),
                        tile_position=(0, 32 * g), skip_group_check=True)
            if "border" not in ablate:
                border(q, S_cur, b_cur, b_new, nf_sb)
            nc.scalar.activation(den_sb[:], pm[:, :], AF.Identity,
                                 bias=b_cur[:, 0:1], scale=1.0)

        def half_bf16_B(q, out_col, mask_sb, den_sb, S_new):
            pt, ps = ptp[q % NTP], psm[q % NSM]
            ptb = pt[:, :].bitcast(BF16)
            if "tr" not in ablate:
                nc.tensor.transpose(ptb[:, 0:128], den_sb[:, 0:128], ident_b[:])
                nc.tensor.transpose(ptb[:, 128:256], den_sb[:, 128:256], ident_b[:])
            tp_src = ptb[:, 0:256].rearrange("p (t g r) -> p t g r", t=2, g=4)[:, :, :, 0]
            nc.vector.reciprocal(rec8[q][:].rearrange("p (t g) -> p t g", t=2), tp_src)
            nc.vector.tensor_mul(out_col[:], rec8[q][:], mask_sb)
            if "sums" not in ablate:
                for g in range(4):
                    nc.tensor.matmul(ps[32 * g:32 * g + 1, :], ones_b[:],
                                     sslice(out_col, g), start=True, stop=True,
                                     tile_position=(0, 32 * g), skip_group_check=True)
                nc.vector.tensor_reduce(S_new[:], ps[:, :], axis=AX.X, op=ALU.add)

        def half_bf16(q, mat, w_col, out_col, mask_sb, den_sb, S_new, S_cur,
                      b_cur, b_new, nf_sb):
            half_bf16_A(q, mat, w_col, mask_sb, den_sb, S_new, S_cur,
                        b_cur, b_new, nf_sb)
            half_bf16_B(q, out_col, mask_sb, den_sb, S_new)

        def half_tail_A(q, mat_hi, mat_lo, whi, wlo, den_sb, S_cur, b_cur,
                        b_new, nf_sb):
            pm = pmv[q % NPM]
            for g in range(4):
                e = q * 4 + g
                for t in range(2):
                    msl = slice(e * 512 + t * 256, e * 512 + (t + 1) * 256)
                    nc.tensor.matmul(pm[32 * g:32 * g + 1, :], wslice(whi, g, t),
                                     mat_hi[:, msl], start=(t == 0), stop=False,
                                     tile_position=(0, 32 * g), skip_group_check=True)
                    nc.tensor.matmul(pm[32 * g:32 * g + 1, :], wslice(wlo, g, t),
                                     mat_hi[:, msl], start=False, stop=False,
                                     tile_position=(0, 32 * g), skip_group_check=True)
                    nc.tensor.matmul(pm[32 * g:32 * g + 1, :], wslice(whi, g, t),
                                     mat_lo[:, msl], start=False, stop=(t == 1),
                                     tile_position=(0, 32 * g), skip_group_check=True)
            border(q, S_cur, b_cur, b_new, nf_sb)
            nc.scalar.activation(den_sb[:], pm[:, :], AF.Identity,
                                 bias=b_cur[:, 0:1], scale=1.0)

        def half_tail_B(q, out_f, out_hi, out_lo, mask_sb, den_sb, S_new):
            pt, ps = ptp[q % NTP], psm[q % NSM]
            nc.tensor.transpose(pt[:, 0:128], den_sb[:, 0:128], ident_f[:])
            nc.tensor.transpose(pt[:, 128:256], den_sb[:, 128:256], ident_f[:])
            tp_src = pt[:, 0:256].rearrange("p (t g r) -> p t g r", t=2, g=4)[:, :, :, 0]
            nc.vector.reciprocal(rec8[q][:].rearrange("p (t g) -> p t g", t=2), tp_src)
            nc.vector.tensor_mul(out_f[:], rec8[q][:], mask_sb)
            nc.scalar.activation(out_hi[:], out_f[:], AF.Copy, bias=0.0, scale=1.0)
            nc.vector.tensor_tensor(out_lo[:], out_f[:], out_hi[:], ALU.subtract)
            for g in range(4):
                nc.tensor.matmul(ps[32 * g:32 * g + 1, :], ones_f[:],
                                 sslice(out_f, g), start=True, stop=True,
                                 tile_position=(0, 32 * g), skip_group_check=True)
            nc.vector.tensor_reduce(S_new[:], ps[:, :], axis=AX.X, op=ALU.add)

        def half_tail(q, mat_hi, mat_lo, whi, wlo, out_f, out_hi, out_lo,
                      mask_sb, den_sb, S_new, S_cur, b_cur, b_new, nf_sb):
            half_tail_A(q, mat_hi, mat_lo, whi, wlo, den_sb, S_cur, b_cur,
                        b_new, nf_sb)
            half_tail_B(q, out_f, out_hi, out_lo, mask_sb, den_sb, S_new)

        def u_args(q):
            return (q, AT_hi, v_col[q], u_col[q],
                    rt01c_sb[:, q * 8:(q + 1) * 8], den_ub[q],
                    Su[q], Sv[q], vb[q], ub[q], ntf_sb[:, q:q + 1])

        def v_args(q):
            return (q, A_hi, u_col[q], v_col[q],
                    ct01c_sb[:, q * 8:(q + 1) * 8], den_vb[q],
                    Sv[q], Su[q], ub[q], vb[q], ndf_sb[:, q:q + 1])

        def u_half(q):
            half_bf16(*u_args(q))

        def v_half(q):
            half_bf16(*v_args(q))

        def stage_A(args):
            (q, mat, w_col, out_col, mask_sb, den_sb, S_new, S_cur,
             b_cur, b_new, nf_sb) = args
            half_bf16_A(q, mat, w_col, mask_sb, den_sb, S_new, S_cur,
                        b_cur, b_new, nf_sb)

        def stage_B(args):
            (q, mat, w_col, out_col, mask_sb, den_sb, S_new, S_cur,
             b_cur, b_new, nf_sb) = args
            half_bf16_B(q, out_col, mask_sb, den_sb, S_new)

        def main_iter(boundaries=False, skewed=True):
            if not skewed:
                h = G // 2
                for q in range(h):
                    u_half(q)
                if boundaries:
                    tc.stage_boundary()
                for q in range(h, G):
                    u_half(q)
                if boundaries:
                    tc.stage_boundary()
                for q in range(h):
                    v_half(q)
                if boundaries:
                    tc.stage_boundary()
                for q in range(h, G):
                    v_half(q)
                return
            # software-pipelined emission: stage A of half k+1 lands on the PE
            # queue before stage B of half k, hiding the ACT->transpose stall
            halves = [u_args(q) for q in range(G)] + [v_args(q) for q in range(G)]
            SKEW = 1
            for k in range(2 * G + SKEW):
                if k < 2 * G:
                    stage_A(halves[k])
                if k >= SKEW:
                    stage_B(halves[k - SKEW])

        if n_main > 0:
            if static:
                for _ in range(n_main):
                    main_iter()
            elif staggered:
                with tc.For_i(0, n_main, 1, staggered_reset=True):
                    main_iter(boundaries=True)
            else:
                with tc.For_i(0, n_main, 1):
                    main_iter()

        for q in range(G):
            nc.vector.memset(v_lo[q][:], 0.0)
            nc.vector.tensor_copy(vf[q][:], v_col[q][:])
        def tu_args(q):
            return (q, AT_hi, AT_lo, v_col[q], v_lo[q], uf[q], u_col[q],
                    u_lo[q], rt01c_sb[:, q * 8:(q + 1) * 8], den_uf[q],
                    Su[q], Sv[q], vb[q], ub[q], ntf_sb[:, q:q + 1])

        def tv_args(q):
            return (q, A_hi, A_lo, u_col[q], u_lo[q], vf[q], v_col[q],
                    v_lo[q], ct01c_sb[:, q * 8:(q + 1) * 8], den_vf[q],
                    Sv[q], Su[q], ub[q], vb[q], ndf_sb[:, q:q + 1])

        def t_stage_A(a):
            (q, mh, ml, whi, wlo, out_f, out_hi, out_lo, mask_sb, den_sb,
             S_new, S_cur, b_cur, b_new, nf_sb) = a
            half_tail_A(q, mh, ml, whi, wlo, den_sb, S_cur, b_cur, b_new, nf_sb)

        def t_stage_B(a):
            (q, mh, ml, whi, wlo, out_f, out_hi, out_lo, mask_sb, den_sb,
             S_new, S_cur, b_cur, b_new, nf_sb) = a
            half_tail_B(q, out_f, out_hi, out_lo, mask_sb, den_sb, S_new)

        for _ in range(n_tail):
            halves = [tu_args(q) for q in range(G)] + [tv_args(q) for q in range(G)]
            t_stage_A(halves[0])
            for k in range(1, 2 * G):
                t_stage_A(halves[k])
                t_stage_B(halves[k - 1])
            t_stage_B(halves[-1])

        ep = P(tc.tile_pool(name="ep", bufs=2))
        for q in range(G):
            vrow = ep.tile([128, 256], F32, tag="vrow", name="vrow")
            ctr = ep.tile([128, 256], F32, tag="ctr", name="ctr")
            nc.sync.dma_start(ctr[:], ct01r[q])
            nc.vector.reciprocal(vrow[:], den_vf[q][:])
            nc.vector.tensor_mul(vrow[:], vrow[:], ctr[:])
            # border scalars to partition 0, then broadcast (gpsimd bcast only
            # works from partition 0 on HW): col 2g = v_b, 2g+1 = u_b
            sc8 = ep.tile([1, 8], F32, tag="sc8", name="sc8")
            for g in range(4):
                nc.vector.tensor_copy(sc8[:, 2 * g:2 * g + 1],
                                      vb[q][32 * g:32 * g + 1, 0:1])
                nc.vector.tensor_copy(sc8[:, 2 * g + 1:2 * g + 2],
                                      ub[q][32 * g:32 * g + 1, 0:1])
            scb = ep.tile([128, 8], F32, tag="scb", name="scb")
            nc.gpsimd.partition_broadcast(scb[:], sc8[:])
            for g in range(4):
                e = q * 4 + g
                vr0 = ep.tile([1, 256], F32, tag="vr0", name="vr0")
                nc.vector.tensor_copy(vr0[:], vrow[32 * g:32 * g + 1, :])
                vbc = ep.tile([128, 256], F32, tag="vbc", name="vbc")
                nc.gpsimd.partition_broadcast(vbc[:], vr0[:])
                Tt = ep.tile([128, 512], F32, tag="Tt", name="Tt")
                rm = ep.tile([128, 2], F32, tag="rm", name="rm")
                bc = ep.tile([128, 2], F32, tag="bc", name="bc")
                for t in range(2):
                    sl = slice(t * 256, (t + 1) * 256)
                    msl = slice(e * 512 + t * 256, e * 512 + (t + 1) * 256)
                    us = wslice(uf[q], g, t)
                    af = ep.tile([128, 256], F32, tag="afrec", name="afrec")
                    nc.vector.tensor_tensor(af[:], A_hi[:, msl], A_lo[:, msl], ALU.add)
                    nc.scalar.activation(Tt[:, sl], af[:], AF.Copy, bias=0.0, scale=us)
                    nc.vector.tensor_mul(Tt[:, sl], Tt[:, sl], vbc[:])
                    nc.vector.tensor_reduce(rm[:, t:t + 1], Tt[:, sl], axis=AX.X,
                                            op=ALU.max)
                    nc.vector.tensor_mul(bc[:, t:t + 1], us, scb[:, 2 * g:2 * g + 1])
                nc.vector.tensor_tensor(rm[:], rm[:], bc[:], ALU.max)
                # colmax (+ border row) broadcast across partitions
                cmb = ep.tile([128, 256], F32, tag="cmb", name="cmb")
                ar2 = ep.tile([128, 256], F32, tag="ar2", name="ar2")
                nc.gpsimd.partition_all_reduce(cmb[:], Tt[:, 0:256], channels=128,
                                               reduce_op=bass_isa.ReduceOp.max)
                nc.gpsimd.partition_all_reduce(ar2[:], Tt[:, 256:512], channels=128,
                                               reduce_op=bass_isa.ReduceOp.max)
                nc.vector.tensor_tensor(cmb[:], cmb[:], ar2[:], ALU.max)
                br = ep.tile([128, 256], F32, tag="br", name="br")
                nc.vector.tensor_scalar_mul(br[:], vbc[:], scb[:, 2 * g + 1:2 * g + 2])
                nc.vector.tensor_tensor(cmb[:], cmb[:], br[:], ALU.max)
                au8 = ep.tile([128, 512], U8, tag="au8", name="au8")
                eq = ep.tile([128, 512], F32, tag="eq", name="eq")
                for t in range(2):
                    sl = slice(t * 256, (t + 1) * 256)
                    nc.vector.tensor_scalar(eq[:, sl], Tt[:, sl], rm[:, t:t + 1],
                                            None, ALU.is_equal)
                    eq2 = ep.tile([128, 256], F32, tag="eq2", name="eq2")
                    nc.vector.tensor_tensor(eq2[:], Tt[:, sl], cmb[:], ALU.is_equal)
                    nc.vector.tensor_mul(eq[:, sl], eq[:, sl], eq2[:])
                    gt = ep.tile([128, 256], F32, tag="gt", name="gt")
                    nc.vector.tensor_scalar(gt[:], Tt[:, sl], 0.0, None, ALU.is_gt)
                    nc.vector.tensor_mul(eq[:, sl], eq[:, sl], gt[:])
                nc.vector.tensor_copy(au8[:], eq[:])
                nc.sync.dma_start(t_out[e], Tt[:])
                nc.sync.dma_start(a_out[e], au8[:])

    nc.compile()
    return nc


def _host_prep(aff_b, nd_b, nt_b):
    n = aff_b.shape[0]
    G = n // 4
    aff_dev = np.ascontiguousarray(
        aff_b.reshape(n, 2, 128, 256).transpose(0, 2, 1, 3).reshape(n, 128, 512))
    afft = np.ascontiguousarray(aff_b.transpose(0, 2, 1))
    afft_dev = np.ascontiguousarray(
        afft.reshape(n, 2, 128, 256).transpose(0, 2, 1, 3).reshape(n, 128, 512))
    p = np.arange(128)
    k = np.arange(2)[:, None] * 128 + p[None, :]
    rt01c = np.zeros((G, 128, 8), np.float32)
    ct01c = np.zeros((G, 128, 8), np.float32)
    ntf = np.zeros((G, 128, 1), np.float32)
    ndf = np.zeros((G, 128, 1), np.float32)
    ct01r = np.zeros((G, 128, 256), np.float32)
    for q in range(G):
        for g in range(4):
            e = q * 4 + g
            for tt in range(2):
                rt01c[q, :, tt * 4 + g] = (k[tt] < nt_b[e])
                ct01c[q, :, tt * 4 + g] = (k[tt] < nd_b[e])
            ntf[q, 32 * g:32 * g + 32, 0] = np.float32(nt_b[e])
            ndf[q, 32 * g:32 * g + 32, 0] = np.float32(nd_b[e])
            ct01r[q, 32 * g, :] = (np.arange(256) < nd_b[e])
    return {
        "aff": aff_dev, "afft": afft_dev, "rt01c": rt01c, "ct01c": ct01c,
        "ntf": ntf, "ndf": ndf, "ct01r": ct01r,
        "ones_in": np.ones((128, 1), np.float32),
    }


def _host_post(t_dev, a_dev, nd_b, nt_b):
    n = t_dev.shape[0]
    T = t_dev.reshape(n, 128, 2, 256).transpose(0, 2, 1, 3).reshape(n, 256, 256)
    A_ = a_dev.reshape(n, 128, 2, 256).transpose(0, 2, 1, 3).reshape(n, 256, 256)
    t_flat = np.zeros((n, MT * MD), np.float32)
    a_flat = np.zeros((n, MT * MD), bool)
    for b in range(n):
        nt, nd = int(nt_b[b]), int(nd_b[b])
        t_flat[b, :nt * nd] = T[b, :nt, :nd].ravel()
        a_flat[b, :nt * nd] = A_[b, :nt, :nd].astype(bool).ravel()
    return t_flat, a_flat


_NC_CACHE = {}


def _get_nc():
    key = (N_EX, N_ITERS, N_TAIL)
    if key not in _NC_CACHE:
        _NC_CACHE[key] = _build(*key)
    return _NC_CACHE[key]


def kernel(affinity_scores, num_detections, num_tracklets):
    aff = np.asarray(affinity_scores, dtype=np.float32)
    nd = np.asarray(num_detections, dtype=np.int32)
    nt = np.asarray(num_tracklets, dtype=np.int32)
    assert aff.shape == (B, MT, MD), aff.shape

    nc = _get_nc()
    in_maps = []
    for c in range(N_CORES):
        s = slice(c * N_EX, (c + 1) * N_EX)
        in_maps.append(_host_prep(aff[s], nd[s], nt[s]))
    res = bass_utils.run_bass_kernel_spmd(nc, in_maps, core_ids=list(range(N_CORES)))

    t_full = np.zeros((B, MT * MD), np.float32)
    a_full = np.zeros((B, MT * MD), bool)
    for c in range(N_CORES):
        s = slice(c * N_EX, (c + 1) * N_EX)
        t_flat, a_flat = _host_post(res.results[c]["t_out"], res.results[c]["a_out"],
                                    nd[s], nt[s])
        t_full[s] = t_flat
        a_full[s] = a_flat
    return t_full, a_full


# revision 14
# speedup vs baseline: 1815.5102x; 1.0571x over previous
"""Trainium2 Bass kernel for nn_AssociationLayer (batched masked Sinkhorn).

Self-contained: kernel(**inputs) takes the FULL unsharded inputs
(affinity_scores [256,256,256] f32, num_detections [256] i32,
num_tracklets [256] i32) and returns (sinkhorn_dense [256, 65536] f32,
assignment_dense [256, 65536] bool), matching the reference.

Distribution: pure data parallelism — batch is sharded 8 x 32 across the 8
NeuronCores; each core solves its 32 examples independently (no collectives).

Per-core algorithm (groups of 4 examples):
  A = exp(10*aff), stored as A_hi + A_lo (two bf16 tiles, ~2^-17 precision)
  95 main iterations: bf16 matvecs on TensorE (4-way column-packed via
    tile_position), denominators moved PSUM->SBUF by ScalarE (+border bias),
    transposed back to weight layout by TensorE, reciprocal+mask on VectorE
  5 tail iterations: 3-term split-bf16 matvecs (f32-grade precision)
  epilogue: transport T = (u_i*A_ij)*v_j, mutual row/col argmax assignment
  Ragged pack to the reference's flattened layout happens on host.
"""
import numpy as np
from contextlib import ExitStack

import concourse.bass as bass
import concourse.bacc as bacc
import concourse.mybir as mybir
from concourse import tile
from concourse import bass_utils
from concourse.masks import make_identity
import concourse.bass_isa as bass_isa

F32 = mybir.dt.float32
BF16 = mybir.dt.bfloat16
U8 = mybir.dt.uint8
AF = mybir.ActivationFunctionType
ALU = mybir.AluOpType
AX = mybir.AxisListType

N_CORES = 8
B, MT, MD = 256, 256, 256
N_EX = B // N_CORES              # 32 examples per core
N_ITERS = 100
N_TAIL = 5


def _build(n_ex=N_EX, n_iters=N_ITERS, n_tail=N_TAIL, static=False, staggered=True, ablate=(), bufs=(3, 3, 2)):
    assert n_ex % 4 == 0
    G = n_ex // 4
    n_main = n_iters - n_tail
    nc = bacc.Bacc("TRN2", target_bir_lowering=False, debug=False)

    aff = nc.dram_tensor("aff", [n_ex, 128, 512], F32, kind="ExternalInput").ap()
    afft = nc.dram_tensor("afft", [n_ex, 128, 512], F32, kind="ExternalInput").ap()
    rt01c = nc.dram_tensor("rt01c", [G, 128, 8], F32, kind="ExternalInput").ap()
    ct01c = nc.dram_tensor("ct01c", [G, 128, 8], F32, kind="ExternalInput").ap()
    ntf = nc.dram_tensor("ntf", [G, 128, 1], F32, kind="ExternalInput").ap()
    ndf = nc.dram_tensor("ndf", [G, 128, 1], F32, kind="ExternalInput").ap()
    ct01r = nc.dram_tensor("ct01r", [G, 128, 256], F32, kind="ExternalInput").ap()
    ones_in = nc.dram_tensor("ones_in", [128, 1], F32, kind="ExternalInput").ap()
    t_out = nc.dram_tensor("t_out", [n_ex, 128, 512], F32, kind="ExternalOutput").ap()
    a_out = nc.dram_tensor("a_out", [n_ex, 128, 512], U8, kind="ExternalOutput").ap()

    with tile.TileContext(nc) as tc, ExitStack() as ctx:
        P = ctx.enter_context
        const = P(tc.tile_pool(name="const", bufs=1))
        mats = P(tc.tile_pool(name="mats", bufs=1))
        state = P(tc.tile_pool(name="state", bufs=1))
        pp_mv = P(tc.tile_pool(name="pp_mv", bufs=1, space="PSUM"))
        pp_tp = P(tc.tile_pool(name="pp_tp", bufs=1, space="PSUM"))
        pp_sm = P(tc.tile_pool(name="pp_sm", bufs=1, space="PSUM"))

        ident_f = const.tile([128, 128], F32, tag="ident_f")
        ident_b = const.tile([128, 128], BF16, tag="ident_b")
        make_identity(nc, ident_f[:])
        nc.vector.tensor_copy(ident_b[:], ident_f[:])
        ones_f = const.tile([128, 1], F32, tag="ones_f")
        ones_b = const.tile([128, 1], BF16, tag="ones_b")
        nc.sync.dma_start(ones_f[:], ones_in[:])
        nc.vector.tensor_copy(ones_b[:], ones_f[:])

        rt01c_sb = const.tile([128, G * 8], F32, tag="rt01c")
        ct01c_sb = const.tile([128, G * 8], F32, tag="ct01c")
        ntf_sb = const.tile([128, G], F32, tag="ntf")
        ndf_sb = const.tile([128, G], F32, tag="ndf")
        for q in range(G):
            nc.sync.dma_start(rt01c_sb[:, q * 8:(q + 1) * 8], rt01c[q])
            nc.sync.dma_start(ct01c_sb[:, q * 8:(q + 1) * 8], ct01c[q])
            nc.sync.dma_start(ntf_sb[:, q:q + 1], ntf[q])
            nc.sync.dma_start(ndf_sb[:, q:q + 1], ndf[q])

        A_hi = mats.tile([128, n_ex * 512], BF16, tag="A_hi")
        A_lo = mats.tile([128, n_ex * 512], BF16, tag="A_lo")
        AT_hi = mats.tile([128, n_ex * 512], BF16, tag="AT_hi")
        AT_lo = mats.tile([128, n_ex * 512], BF16, tag="AT_lo")
        with tc.tile_pool(name="stage", bufs=3) as stg:
            for e in range(n_ex):
                for src, hi, lo in ((aff, A_hi, A_lo), (afft, AT_hi, AT_lo)):
                    sl = slice(e * 512, (e + 1) * 512)
                    st = stg.tile([128, 512], F32, tag="ldstage", name="ldstage")
                    nc.sync.dma_start(st[:], src[e])
                    af = stg.tile([128, 512], F32, tag="af32", name="af32")
                    nc.scalar.activation(af[:], st[:], AF.Exp, bias=0.0, scale=10.0)
                    nc.scalar.activation(hi[:, sl], af[:], AF.Copy, bias=0.0, scale=1.0)
                    nc.vector.tensor_tensor(lo[:, sl], af[:], hi[:, sl], ALU.subtract)

        def tiles(shape, dt_, pfx):
            return [state.tile(shape, dt_, tag=f"{pfx}{q}", name=f"{pfx}{q}")
                    for q in range(G)]
        u_col = tiles([128, 8], BF16, "u")
        v_col = tiles([128, 8], BF16, "v")
        uf = tiles([128, 8], F32, "uf")
        vf = tiles([128, 8], F32, "vf")
        u_lo = tiles([128, 8], BF16, "ulo")
        v_lo = tiles([128, 8], BF16, "vlo")
        den_ub = tiles([128, 256], BF16, "dub")
        den_vb = tiles([128, 256], BF16, "dvb")
        den_uf = tiles([128, 256], F32, "duf")
        den_vf = tiles([128, 256], F32, "dvf")
        Su = tiles([128, 1], F32, "Su")
        Sv = tiles([128, 1], F32, "Sv")
        ub = tiles([128, 1], F32, "ub")
        vb = tiles([128, 1], F32, "vb")
        rec8 = tiles([128, 8], F32, "r8")
        dbt = tiles([128, 1], F32, "db")

        NPM, NTP, NSM = bufs
        pmv = [pp_mv.tile([128, 256], F32, tag=f"pmv{i}", name=f"pmv{i}")
               for i in range(NPM)]
        ptp = [pp_tp.tile([128, 256], F32, tag=f"ptp{i}", name=f"ptp{i}")
               for i in range(NTP)]
        psm = [pp_sm.tile([128, 2], F32, tag=f"psm{i}", name=f"psm{i}")
               for i in range(NSM)]
        for t_ in pmv + ptp + psm:
            nc.vector.memset(t_[:, :], 0.0)

        for q in range(G):
            nc.vector.tensor_copy(v_col[q][:], ct01c_sb[:, q * 8:(q + 1) * 8])
            nc.vector.memset(vb[q][:], 1.0)
            nc.vector.tensor_copy(Sv[q][:], ndf_sb[:, q:q + 1])

        def wslice(col, g, t):
            return col[:, :].rearrange("p (t g) -> p g t", g=4)[:, g, t:t + 1]

        def sslice(col, g):
            return col[:, :].rearrange("p (t g) -> p g t", g=4)[:, g, :]

        def border(q, S_cur, b_cur, b_new, nf_sb):
            nc.vector.tensor_add(dbt[q][:], S_cur[:], b_cur[:])
            nc.vector.reciprocal(dbt[q][:], dbt[q][:])
            nc.vector.tensor_mul(b_new[:], nf_sb, dbt[q][:])

        def half_bf16_A(q, mat, w_col, mask_sb, den_sb, S_new, S_cur,
                        b_cur, b_new, nf_sb):
            pm = pmv[q % NPM]
            for g in range(4):
                e = q * 4 + g
                for t in range(2):
                    nc.tensor.matmul(
                        pm[32 * g:32 * g + 1, :], wslice(w_col, g, t),
                        mat[:, e * 512 + t * 256: e * 512 + (t + 1) * 256],
                        start=(t == 0), stop=(t == 1),
                        tile_position=(0, 32 * g), skip_group_check=True)
            if "border" not in ablate:
                border(q, S_cur, b_cur, b_new, nf_sb)
            nc.scalar.activation(den_sb[:], pm[:, :], AF.Identity,
                                 bias=b_cur[:, 0:1], scale=1.0)

        def half_bf16_B(q, out_col, mask_sb, den_sb, S_new):
            pt, ps = ptp[q % NTP], psm[q % NSM]
            ptb = pt[:, :].bitcast(BF16)
            if "tr" not in ablate:
                nc.tensor.transpose(ptb[:, 0:128], den_sb[:, 0:128], ident_b[:])
                nc.tensor.transpose(ptb[:, 128:256], den_sb[:, 128:256], ident_b[:])
            tp_src = ptb[:, 0:256].rearrange("p (t g r) -> p t g r", t=2, g=4)[:, :, :, 0]
            nc.vector.reciprocal(rec8[q][:].rearrange("p (t g) -> p t g", t=2), tp_src)
            nc.vector.tensor_mul(out_col[:], rec8[q][:], mask_sb)
            if "sums" not in ablate:
                for g in range(4):
                    nc.tensor.matmul(ps[32 * g:32 * g + 1, :], ones_b[:],
                                     sslice(out_col, g), start=True, stop=True,
                                     tile_position=(0, 32 * g), skip_group_check=True)
                nc.vector.tensor_reduce(S_new[:], ps[:, :], axis=AX.X, op=ALU.add)

        def half_bf16(q, mat, w_col, out_col, mask_sb, den_sb, S_new, S_cur,
                      b_cur, b_new, nf_sb):
            half_bf16_A(q, mat, w_col, mask_sb, den_sb, S_new, S_cur,
                        b_cur, b_new, nf_sb)
            half_bf16_B(q, out_col, mask_sb, den_sb, S_new)

        def half_tail_A(q, mat_hi, mat_lo, whi, wlo, den_sb, S_cur, b_cur,
                        b_new, nf_sb):
            pm = pmv[q % NPM]
            for g in range(4):
                e = q * 4 + g
                for t in range(2):
                    msl = slice(e * 512 + t * 256, e * 512 + (t + 1) * 256)
                    nc.tensor.matmul(pm[32 * g:32 * g + 1, :], wslice(whi, g, t),
                                     mat_hi[:, msl], start=(t == 0), stop=False,
                                     tile_position=(0, 32 * g), skip_group_check=True)
                    nc.tensor.matmul(pm[32 * g:32 * g + 1, :], wslice(wlo, g, t),
                                     mat_hi[:, msl], start=False, stop=False,
                                     tile_position=(0, 32 * g), skip_group_check=True)
                    nc.tensor.matmul(pm[32 * g:32 * g + 1, :], wslice(whi, g, t),
                                     mat_lo[:, msl], start=False, stop=(t == 1),
                                     tile_position=(0, 32 * g), skip_group_check=True)
            border(q, S_cur, b_cur, b_new, nf_sb)
            nc.scalar.activation(den_sb[:], pm[:, :], AF.Identity,
                                 bias=b_cur[:, 0:1], scale=1.0)

        def half_tail_B(q, out_f, out_hi, out_lo, mask_sb, den_sb, S_new):
            pt, ps = ptp[q % NTP], psm[q % NSM]
            nc.tensor.transpose(pt[:, 0:128], den_sb[:, 0:128], ident_f[:])
            nc.tensor.transpose(pt[:, 128:256], den_sb[:, 128:256], ident_f[:])
            tp_src = pt[:, 0:256].rearrange("p (t g r) -> p t g r", t=2, g=4)[:, :, :, 0]
            nc.vector.reciprocal(rec8[q][:].rearrange("p (t g) -> p t g", t=2), tp_src)
            nc.vector.tensor_mul(out_f[:], rec8[q][:], mask_sb)
            nc.scalar.activation(out_hi[:], out_f[:], AF.Copy, bias=0.0, scale=1.0)
            nc.vector.tensor_tensor(out_lo[:], out_f[:], out_hi[:], ALU.subtract)
            for g in range(4):
                nc.tensor.matmul(ps[32 * g:32 * g + 1, :], ones_f[:],
                                 sslice(out_f, g), start=True, stop=True,
                                 tile_position=(0, 32 * g), skip_group_check=True)
            nc.vector.tensor_reduce(S_new[:], ps[:, :], axis=AX.X, op=ALU.add)

        def half_tail(q, mat_hi, mat_lo, whi, wlo, out_f, out_hi, out_lo,
                      mask_sb, den_sb, S_new, S_cur, b_cur, b_new, nf_sb):
            half_tail_A(q, mat_hi, mat_lo, whi, wlo, den_sb, S_cur, b_cur,
                        b_new, nf_sb)
            half_tail_B(q, out_f, out_hi, out_lo, mask_sb, den_sb, S_new)

        def u_args(q):
            return (q, AT_hi, v_col[q], u_col[q],
                    rt01c_sb[:, q * 8:(q + 1) * 8], den_ub[q],
                    Su[q], Sv[q], vb[q], ub[q], ntf_sb[:, q:q + 1])

        def v_args(q):
            return (q, A_hi, u_col[q], v_col[q],
                    ct01c_sb[:, q * 8:(q + 1) * 8], den_vb[q],
                    Sv[q], Su[q], ub[q], vb[q], ndf_sb[:, q:q + 1])

        def u_half(q):
            half_bf16(*u_args(q))

        def v_half(q):
            half_bf16(*v_args(q))

        def stage_A(args):
            (q, mat, w_col, out_col, mask_sb, den_sb, S_new, S_cur,
             b_cur, b_new, nf_sb) = args
            half_bf16_A(q, mat, w_col, mask_sb, den_sb, S_new, S_cur,
                        b_cur, b_new, nf_sb)

        def stage_B(args):
            (q, mat, w_col, out_col, mask_sb, den_sb, S_new, S_cur,
             b_cur, b_new, nf_sb) = args
            half_bf16_B(q, out_col, mask_sb, den_sb, S_new)

        def main_iter(boundaries=False, skewed=True):
            if not skewed:
                h = G // 2
                for q in range(h):
                    u_half(q)
                if boundaries:
                    tc.stage_boundary()
                for q in range(h, G):
                    u_half(q)
                if boundaries:
                    tc.stage_boundary()
                for q in range(h):
                    v_half(q)
                if boundaries:
                    tc.stage_boundary()
                for q in range(h, G):
                    v_half(q)
                return
            # software-pipelined emission: stage A of half k+1 lands on the PE
            # queue before stage B of half k, hiding the ACT->transpose stall
            halves = [u_args(q) for q in range(G)] + [v_args(q) for q in range(G)]
            SKEW = 1
            nb = (2 * G) // 4
            for k in range(2 * G + SKEW):
                if boundaries and k in (nb, 2 * nb, 3 * nb):
                    tc.stage_boundary()
                if k < 2 * G:
                    stage_A(halves[k])
                if k >= SKEW:
                    stage_B(halves[k - SKEW])

        if n_main > 0:
            if static:
                for _ in range(n_main):
                    main_iter()
            elif staggered:
                with tc.For_i(0, n_main, 1, staggered_reset=True):
                    main_iter(boundaries=True)
            else:
                with tc.For_i(0, n_main, 1):
                    main_iter()

        for q in range(G):
            nc.vector.memset(v_lo[q][:], 0.0)
            nc.vector.tensor_copy(vf[q][:], v_col[q][:])
        def tu_args(q):
            return (q, AT_hi, AT_lo, v_col[q], v_lo[q], uf[q], u_col[q],
                    u_lo[q], rt01c_sb[:, q * 8:(q + 1) * 8], den_uf[q],
                    Su[q], Sv[q], vb[q], ub[q], ntf_sb[:, q:q + 1])

        def tv_args(q):
            return (q, A_hi, A_lo, u_col[q], u_lo[q], vf[q], v_col[q],
                    v_lo[q], ct01c_sb[:, q * 8:(q + 1) * 8], den_vf[q],
                    Sv[q], Su[q], ub[q], vb[q], ndf_sb[:, q:q + 1])

        def t_stage_A(a):
            (q, mh, ml, whi, wlo, out_f, out_hi, out_lo, mask_sb, den_sb,
             S_new, S_cur, b_cur, b_new, nf_sb) = a
            half_tail_A(q, mh, ml, whi, wlo, den_sb, S_cur, b_cur, b_new, nf_sb)

        def t_stage_B(a):
            (q, mh, ml, whi, wlo, out_f, out_hi, out_lo, mask_sb, den_sb,
             S_new, S_cur, b_cur, b_new, nf_sb) = a
            half_tail_B(q, out_f, out_hi, out_lo, mask_sb, den_sb, S_new)

        for _ in range(n_tail):
            halves = [tu_args(q) for q in range(G)] + [tv_args(q) for q in range(G)]
            t_stage_A(halves[0])
            for k in range(1, 2 * G):
                t_stage_A(halves[k])
                t_stage_B(halves[k - 1])
            t_stage_B(halves[-1])

        ep = P(tc.tile_pool(name="ep", bufs=2))
        for q in range(G):
            vrow = ep.tile([128, 256], F32, tag="vrow", name="vrow")
            ctr = ep.tile([128, 256], F32, tag="ctr", name="ctr")
            nc.sync.dma_start(ctr[:], ct01r[q])
            nc.vector.reciprocal(vrow[:], den_vf[q][:])
            nc.vector.tensor_mul(vrow[:], vrow[:], ctr[:])
            # border scalars to partition 0, then broadcast (gpsimd bcast only
            # works from partition 0 on HW): col 2g = v_b, 2g+1 = u_b
            sc8 = ep.tile([1, 8], F32, tag="sc8", name="sc8")
            for g in range(4):
                nc.vector.tensor_copy(sc8[:, 2 * g:2 * g + 1],
                                      vb[q][32 * g:32 * g + 1, 0:1])
                nc.vector.tensor_copy(sc8[:, 2 * g + 1:2 * g + 2],
                                      ub[q][32 * g:32 * g + 1, 0:1])
            scb = ep.tile([128, 8], F32, tag="scb", name="scb")
            nc.gpsimd.partition_broadcast(scb[:], sc8[:])
            for g in range(4):
                e = q * 4 + g
                vr0 = ep.tile([1, 256], F32, tag="vr0", name="vr0")
                nc.vector.tensor_copy(vr0[:], vrow[32 * g:32 * g + 1, :])
                vbc = ep.tile([128, 256], F32, tag="vbc", name="vbc")
                nc.gpsimd.partition_broadcast(vbc[:], vr0[:])
                Tt = ep.tile([128, 512], F32, tag="Tt", name="Tt")
                rm = ep.tile([128, 2], F32, tag="rm", name="rm")
                bc = ep.tile([128, 2], F32, tag="bc", name="bc")
                for t in range(2):
                    sl = slice(t * 256, (t + 1) * 256)
                    msl = slice(e * 512 + t * 256, e * 512 + (t + 1) * 256)
                    us = wslice(uf[q], g, t)
                    af = ep.tile([128, 256], F32, tag="afrec", name="afrec")
                    nc.vector.tensor_tensor(af[:], A_hi[:, msl], A_lo[:, msl], ALU.add)
                    nc.scalar.activation(Tt[:, sl], af[:], AF.Copy, bias=0.0, scale=us)
                    nc.vector.tensor_mul(Tt[:, sl], Tt[:, sl], vbc[:])
                    nc.vector.tensor_reduce(rm[:, t:t + 1], Tt[:, sl], axis=AX.X,
                                            op=ALU.max)
                    nc.vector.tensor_mul(bc[:, t:t + 1], us, scb[:, 2 * g:2 * g + 1])
                nc.vector.tensor_tensor(rm[:], rm[:], bc[:], ALU.max)
                # colmax (+ border row) broadcast across partitions
                cmb = ep.tile([128, 256], F32, tag="cmb", name="cmb")
                ar2 = ep.tile([128, 256], F32, tag="ar2", name="ar2")
                nc.gpsimd.partition_all_reduce(cmb[:], Tt[:, 0:256], channels=128,
                                               reduce_op=bass_isa.ReduceOp.max)
                nc.gpsimd.partition_all_reduce(ar2[:], Tt[:, 256:512], channels=128,
                                               reduce_op=bass_isa.ReduceOp.max)
                nc.vector.tensor_tensor(cmb[:], cmb[:], ar2[:], ALU.max)
                br = ep.tile([128, 256], F32, tag="br", name="br")
                nc.vector.tensor_scalar_mul(br[:], vbc[:], scb[:, 2 * g + 1:2 * g + 2])
                nc.vector.tensor_tensor(cmb[:], cmb[:], br[:], ALU.max)
                au8 = ep.tile([128, 512], U8, tag="au8", name="au8")
                eq = ep.tile([128, 512], F32, tag="eq", name="eq")
                for t in range(2):
                    sl = slice(t * 256, (t + 1) * 256)
                    nc.vector.tensor_scalar(eq[:, sl], Tt[:, sl], rm[:, t:t + 1],
                                            None, ALU.is_equal)
                    eq2 = ep.tile([128, 256], F32, tag="eq2", name="eq2")
                    nc.vector.tensor_tensor(eq2[:], Tt[:, sl], cmb[:], ALU.is_equal)
                    nc.vector.tensor_mul(eq[:, sl], eq[:, sl], eq2[:])
                    gt = ep.tile([128, 256], F32, tag="gt", name="gt")
                    nc.vector.tensor_scalar(gt[:], Tt[:, sl], 0.0, None, ALU.is_gt)
                    nc.vector.tensor_mul(eq[:, sl], eq[:, sl], gt[:])
                nc.vector.tensor_copy(au8[:], eq[:])
                nc.sync.dma_start(t_out[e], Tt[:])
                nc.sync.dma_start(a_out[e], au8[:])

    nc.compile()
    return nc


def _host_prep(aff_b, nd_b, nt_b):
    n = aff_b.shape[0]
    G = n // 4
    aff_dev = np.ascontiguousarray(
        aff_b.reshape(n, 2, 128, 256).transpose(0, 2, 1, 3).reshape(n, 128, 512))
    afft = np.ascontiguousarray(aff_b.transpose(0, 2, 1))
    afft_dev = np.ascontiguousarray(
        afft.reshape(n, 2, 128, 256).transpose(0, 2, 1, 3).reshape(n, 128, 512))
    p = np.arange(128)
    k = np.arange(2)[:, None] * 128 + p[None, :]
    rt01c = np.zeros((G, 128, 8), np.float32)
    ct01c = np.zeros((G, 128, 8), np.float32)
    ntf = np.zeros((G, 128, 1), np.float32)
    ndf = np.zeros((G, 128, 1), np.float32)
    ct01r = np.zeros((G, 128, 256), np.float32)
    for q in range(G):
        for g in range(4):
            e = q * 4 + g
            for tt in range(2):
                rt01c[q, :, tt * 4 + g] = (k[tt] < nt_b[e])
                ct01c[q, :, tt * 4 + g] = (k[tt] < nd_b[e])
            ntf[q, 32 * g:32 * g + 32, 0] = np.float32(nt_b[e])
            ndf[q, 32 * g:32 * g + 32, 0] = np.float32(nd_b[e])
            ct01r[q, 32 * g, :] = (np.arange(256) < nd_b[e])
    return {
        "aff": aff_dev, "afft": afft_dev, "rt01c": rt01c, "ct01c": ct01c,
        "ntf": ntf, "ndf": ndf, "ct01r": ct01r,
        "ones_in": np.ones((128, 1), np.float32),
    }


def _host_post(t_dev, a_dev, nd_b, nt_b):
    n = t_dev.shape[0]
    T = t_dev.reshape(n, 128, 2, 256).transpose(0, 2, 1, 3).reshape(n, 256, 256)
    A_ = a_dev.reshape(n, 128, 2, 256).transpose(0, 2, 1, 3).reshape(n, 256, 256)
    t_flat = np.zeros((n, MT * MD), np.float32)
    a_flat = np.zeros((n, MT * MD), bool)
    for b in range(n):
        nt, nd = int(nt_b[b]), int(nd_b[b])
        t_flat[b, :nt * nd] = T[b, :nt, :nd].ravel()
        a_flat[b, :nt * nd] = A_[b, :nt, :nd].astype(bool).ravel()
    return t_flat, a_flat


_NC_CACHE = {}


def _get_nc():
    key = (N_EX, N_ITERS, N_TAIL)
    if key not in _NC_CACHE:
        _NC_CACHE[key] = _build(*key)
    return _NC_CACHE[key]


def kernel(affinity_scores, num_detections, num_tracklets):
    aff = np.asarray(affinity_scores, dtype=np.float32)
    nd = np.asarray(num_detections, dtype=np.int32)
    nt = np.asarray(num_tracklets, dtype=np.int32)
    assert aff.shape == (B, MT, MD), aff.shape

    nc = _get_nc()
    in_maps = []
    for c in range(N_CORES):
        s = slice(c * N_EX, (c + 1) * N_EX)
        in_maps.append(_host_prep(aff[s], nd[s], nt[s]))
    res = bass_utils.run_bass_kernel_spmd(nc, in_maps, core_ids=list(range(N_CORES)))

    t_full = np.zeros((B, MT * MD), np.float32)
    a_full = np.zeros((B, MT * MD), bool)
    for c in range(N_CORES):
        s = slice(c * N_EX, (c + 1) * N_EX)
        t_flat, a_flat = _host_post(res.results[c]["t_out"], res.results[c]["a_out"],
                                    nd[s], nt[s])
        t_full[s] = t_flat
        a_full[s] = a_flat
    return t_full, a_full
